# revision 4
# baseline (speedup 1.0000x reference)
"""Trainium2 Bass kernel for nn_EncoderBlock (B=4, N=2048, C=1024, H=8).

Sharding: 8 cores = (batch, token-half). Core c handles batch c//2 and owns
1024 query tokens (half c%2). k/v are computed over the full 2048 tokens of
the batch on each core (duplicated between the 2 cores of a batch) so there
are no collectives. The host rotates each core's transposed batch so its own
tokens always sit at columns 0:1024 -> identical SPMD program on all cores
(softmax over keys is permutation invariant).

On-chip layout is transposed throughout: [feature(partition), token(free)].
Cross-partition reductions (LN stats, softmax sums) use an all-ones [128,128]
stationary matmul, which also pre-broadcasts the result across partitions.
v is produced in [token, feature] layout straight from the qkv matmul (hT as
the stationary operand) so attention needs no on-chip transposes at all.

All matmuls run in bf16 (fp32 PSUM accumulation); elementwise math in fp32.
"""

import numpy as np
import ml_dtypes

import concourse.bass as bass
import concourse.tile as tile
from concourse import bacc, mybir
from concourse.bass_utils import run_bass_kernel_spmd

F32 = mybir.dt.float32
BF16 = mybir.dt.bfloat16
AF = mybir.ActivationFunctionType
ALU = mybir.AluOpType

B, N, C, H, D = 4, 2048, 1024, 8, 128
NT = 2048          # tokens per batch (k/v extent)
NO = 1024          # own (query) tokens per core
CT = C // 128      # 8 c-tiles
SCALE = float(D) ** -0.5
EPS = 1e-5
HG = 2             # heads per group
NGRP = H // HG     # 4 head groups
N_CORES = 8

# vecs packing order (columns of the [C, 9] per-feature constant table)
V_LN1G, V_LN1B, V_LN2G, V_LN2B, V_BPROJ, V_B1, V_B2, V_BNS, V_BNB = range(9)


def emit_body(nc, tc, ctx, pools, dram):
    (pconst, pmean, pxch, pxt, phT, pqkv, pw, pscr, px2, ph1, pxo, poT,
     psA, psS, psR, psO) = pools
    xT, wqkvT, wprojT, w1T, w2T, vecs, ones, outT = dram

    # ---- constants ----
    vecs_sb = pconst.tile([128, CT, 9], F32, name="vecs_sb")
    nc.sync.dma_start(vecs_sb[:], vecs.rearrange("(o p) k -> p o k", p=128))
    ones_sb = pconst.tile([128, 128], BF16, name="ones_sb")
    nc.sync.dma_start(ones_sb[:], ones[:])
    eps_sb = pconst.tile([128, 1], F32, name="eps_sb")
    nc.vector.memset(eps_sb[:], EPS)

    def vcol(ct, k):
        return vecs_sb[:, ct, k : k + 1]

    # ---- LN1 stats: colsum(x) and colsum(x^2) over features, per 512-token chunk
    mean_b = pmean.tile([128, NT], F32, name="mean_b", tag="mean", bufs=2)
    rstd_b = pmean.tile([128, NT], F32, name="rstd_b", tag="mean", bufs=2)
    for ch in range(4):
        sl = slice(ch * 512, (ch + 1) * 512)
        ps1 = psA.tile([128, 512], F32, name="ps1", tag="psA", bufs=2)
        ps2 = psA.tile([128, 512], F32, name="ps2", tag="psA", bufs=2)
        for c in range(CT):
            xch = pxch.tile([128, 512], F32, name="xch", tag="xch", bufs=2)
            nc.sync.dma_start(xch[:], xT[c * 128 : (c + 1) * 128, sl])
            xcb = pscr.tile([128, 512], BF16, name="xcb", tag="cst", bufs=4)
            nc.gpsimd.tensor_copy(xcb[:], xch[:])
            sqb = pscr.tile([128, 512], BF16, name="sqb", tag="cst", bufs=4)
            nc.scalar.square(sqb[:], xch[:])
            nc.tensor.matmul(ps1[:], ones_sb[:], xcb[:], start=(c == 0), stop=(c == CT - 1))
            nc.tensor.matmul(ps2[:], ones_sb[:], sqb[:], start=(c == 0), stop=(c == CT - 1))
        # mean = s1/C ; msq = s2/C ; var = msq - mean^2 ; rstd = 1/sqrt(var+eps)
        nc.scalar.mul(mean_b[:, sl], ps1[:], 1.0 / C)
        nc.scalar.mul(rstd_b[:, sl], ps2[:], 1.0 / C)  # msq parked in rstd_b
    tsq = pscr.tile([128, NT], F32, name="tsq", tag="sc2k", bufs=2)
    nc.vector.tensor_mul(tsq[:], mean_b[:], mean_b[:])
    nc.vector.tensor_sub(rstd_b[:], rstd_b[:], tsq[:])  # var
    nc.scalar.activation(rstd_b[:], rstd_b[:], AF.Sqrt, bias=eps_sb[:], scale=1.0)
    nc.vector.reciprocal(rstd_b[:], rstd_b[:])

    # ---- LN1 normalize -> hT (bf16, [c-tile][128, 2048]) ----
    hT = []
    for c in range(CT):
        xt_c = pxt.tile([128, NT], F32, name="xt_c", tag="xt", bufs=2)
        nc.sync.dma_start(xt_c[:], xT[c * 128 : (c + 1) * 128, :])
        t = pscr.tile([128, NT], F32, name="t", tag="sc2k", bufs=2)
        nc.vector.tensor_sub(t[:], xt_c[:], mean_b[:])
        t2 = pscr.tile([128, NT], F32, name="t2", tag="sc2k", bufs=2)
        nc.vector.scalar_tensor_tensor(
            out=t2[:], in0=t[:], scalar=vcol(c, V_LN1G), in1=rstd_b[:],
            op0=ALU.mult, op1=ALU.mult,
        )
        h_c = phT.tile([128, NT], BF16, name="h_c", tag="hT", bufs=8)
        nc.scalar.activation(h_c[:], t2[:], AF.Identity, bias=vcol(c, V_LN1B), scale=1.0)
        hT.append(h_c)

    # ---- per head-group: qkv then attention ----
    oT = []
    for c in range(CT):
        o_c = poT.tile([128, NO], BF16, name="o_c", tag="oT", bufs=8)
        oT.append(o_c)

    for g in range(NGRP):
        q0 = g * HG * 128          # q feature offset of this group
        k0 = C + g * HG * 128      # k feature offset
        v0 = 2 * C + g * HG * 128  # v feature offset
        wq, wk, wv = [], [], []
        for c in range(CT):
            csl = slice(c * 128, (c + 1) * 128)
            wq_c = pw.tile([128, HG * 128], BF16, name="wq_c", tag="w", bufs=24)
            nc.sync.dma_start(wq_c[:], wqkvT[csl, q0 : q0 + HG * 128])
            wk_c = pw.tile([128, HG * 128], BF16, name="wk_c", tag="w", bufs=24)
            nc.sync.dma_start(wk_c[:], wqkvT[csl, k0 : k0 + HG * 128])
            wv_c = pw.tile([128, HG * 128], BF16, name="wv_c", tag="w", bufs=24)
            nc.sync.dma_start(wv_c[:], wqkvT[csl, v0 : v0 + HG * 128])
            wq.append(wq_c); wk.append(wk_c); wv.append(wv_c)

        kT, qT = [], []
        for hl in range(HG):
            fsl = slice(hl * 128, (hl + 1) * 128)
            kT_h = pqkv.tile([128, NT], BF16, name="kT_h", tag="kT", bufs=2)
            for jc in range(4):
                jsl = slice(jc * 512, (jc + 1) * 512)
                ps = psA.tile([128, 512], F32, name="psk", tag="psA", bufs=2)
                for c in range(CT):
                    nc.tensor.matmul(ps[:], wk[c][:, fsl], hT[c][:, jsl],
                                     start=(c == 0), stop=(c == CT - 1))
                nc.scalar.activation(kT_h[:, jsl], ps[:], AF.Copy, bias=0.0, scale=1.0)
            kT.append(kT_h)
            qT_h = pqkv.tile([128, NO], BF16, name="qT_h", tag="qT", bufs=2)
            for ic in range(2):
                isl = slice(ic * 512, (ic + 1) * 512)
                ps = psA.tile([128, 512], F32, name="psq", tag="psA", bufs=2)
                for c in range(CT):
                    nc.tensor.matmul(ps[:], wq[c][:, fsl], hT[c][:, isl],
                                     start=(c == 0), stop=(c == CT - 1))
                nc.vector.tensor_copy(qT_h[:, isl], ps[:])
            qT.append(qT_h)

        vv = []
        for j in range(16):
            jsl = slice(j * 128, (j + 1) * 128)
            ps = psA.tile([128, HG * 128], F32, name="psv", tag="psA", bufs=2)
            for c in range(CT):
                nc.tensor.matmul(ps[:], hT[c][:, jsl], wv[c][:],
                                 start=(c == 0), stop=(c == CT - 1))
            v_j = pqkv.tile([128, HG * 128], BF16, name="v_j", tag="vv", bufs=16)
            nc.vector.tensor_copy(v_j[:], ps[:])
            vv.append(v_j)

        for hl in range(HG):
            fsl = slice(hl * 128, (hl + 1) * 128)
            for ic in range(2):
                isl = slice(ic * 512, (ic + 1) * 512)
                ps_sum = psR.tile([128, 512], F32, name="ps_sum", tag="psR", bufs=2)
                ps_o = psO.tile([128, 512], F32, name="ps_o", tag="psO", bufs=2)
                for j in range(16):
                    ps_sc = psS.tile([128, 512], F32, name="ps_sc", tag="psS", bufs=2)
                    nc.tensor.matmul(ps_sc[:], kT[hl][:, j * 128 : (j + 1) * 128],
                                     qT[hl][:, isl], start=True, stop=True)
                    e_j = pscr.tile([128, 512], BF16, name="e_j", tag="expT", bufs=4)
                    nc.scalar.activation(e_j[:], ps_sc[:], AF.Exp, bias=0.0, scale=SCALE)
                    nc.tensor.matmul(ps_sum[:], ones_sb[:], e_j[:],
                                     start=(j == 0), stop=(j == 15))
                    nc.tensor.matmul(ps_o[:], vv[j][:, fsl], e_j[:],
                                     start=(j == 0), stop=(j == 15))
                rc = pscr.tile([128, 512], F32, name="rc", tag="rc", bufs=2)
                nc.vector.reciprocal(rc[:], ps_sum[:])
                nc.vector.tensor_mul(oT[g * HG + hl][:, isl], ps_o[:], rc[:])

    # ---- tail (proj + residual, LN2, fc1, fc2, BN, residual), per 512-token chunk
    for t2c in range(2):
        tsl = slice(t2c * 512, (t2c + 1) * 512)

        wp = []
        for c in range(CT):
            wp_c = pw.tile([128, C], BF16, name="wp_c", tag="wfull", bufs=8)
            nc.sync.dma_start(wp_c[:], wprojT[c * 128 : (c + 1) * 128, :])
            wp.append(wp_c)
        x2 = []
        for ft in range(CT):
            ps = psA.tile([128, 512], F32, name="psp", tag="psA", bufs=2)
            for c in range(CT):
                nc.tensor.matmul(ps[:], wp[c][:, ft * 128 : (ft + 1) * 128],
                                 oT[c][:, tsl], start=(c == 0), stop=(c == CT - 1))
            xo = pxo.tile([128, 512], F32, name="xo", tag="xo", bufs=2)
            nc.sync.dma_start(xo[:], xT[ft * 128 : (ft + 1) * 128, tsl])
            x2_ft = px2.tile([128, 512], F32, name="x2_ft", tag="x2", bufs=8)
            nc.vector.scalar_tensor_tensor(
                out=x2_ft[:], in0=ps[:], scalar=vcol(ft, V_BPROJ), in1=xo[:],
                op0=ALU.add, op1=ALU.add,
            )
            x2.append(x2_ft)

        # LN2 on the 512-token chunk
        mean2 = pmean.tile([128, 512], F32, name="mean2", tag="mean", bufs=2)
        rstd2 = pmean.tile([128, 512], F32, name="rstd2", tag="mean", bufs=2)
        ps1 = psA.tile([128, 512], F32, name="ps1b", tag="psA", bufs=2)
        ps2 = psA.tile([128, 512], F32, name="ps2b", tag="psA", bufs=2)
        for c in range(CT):
            xcb = pscr.tile([128, 512], BF16, name="xcb2", tag="cst", bufs=4)
            nc.gpsimd.tensor_copy(xcb[:], x2[c][:])
            sqb = pscr.tile([128, 512], BF16, name="sqb2", tag="cst", bufs=4)
            nc.scalar.square(sqb[:], x2[c][:])
            nc.tensor.matmul(ps1[:], ones_sb[:], xcb[:], start=(c == 0), stop=(c == CT - 1))
            nc.tensor.matmul(ps2[:], ones_sb[:], sqb[:], start=(c == 0), stop=(c == CT - 1))
        nc.scalar.mul(mean2[:], ps1[:], 1.0 / C)
        nc.scalar.mul(rstd2[:], ps2[:], 1.0 / C)
        t2q = pscr.tile([128, 512], F32, name="t2q", tag="rc", bufs=2)
        nc.vector.tensor_mul(t2q[:], mean2[:], mean2[:])
        nc.vector.tensor_sub(rstd2[:], rstd2[:], t2q[:])
        nc.scalar.activation(rstd2[:], rstd2[:], AF.Sqrt, bias=eps_sb[:], scale=1.0)
        nc.vector.reciprocal(rstd2[:], rstd2[:])

        ln2 = []
        for c in range(CT):
            t = pscr.tile([128, 512], F32, name="tn2", tag="er", bufs=4)
            nc.vector.tensor_sub(t[:], x2[c][:], mean2[:])
            t2 = pscr.tile([128, 512], F32, name="tn2b", tag="er", bufs=4)
            nc.vector.scalar_tensor_tensor(
                out=t2[:], in0=t[:], scalar=vcol(c, V_LN2G), in1=rstd2[:],
                op0=ALU.mult, op1=ALU.mult,
            )
            ln2_c = phT.tile([128, 512], BF16, name="ln2_c", tag="hT", bufs=8)
            nc.scalar.activation(ln2_c[:], t2[:], AF.Identity, bias=vcol(c, V_LN2B), scale=1.0)
            ln2.append(ln2_c)

        w1 = []
        for c in range(CT):
            w1_c = pw.tile([128, C], BF16, name="w1_c", tag="wfull", bufs=8)
            nc.sync.dma_start(w1_c[:], w1T[c * 128 : (c + 1) * 128, :])
            w1.append(w1_c)
        h1 = []
        for ft in range(CT):
            ps = psA.tile([128, 512], F32, name="psf1", tag="psA", bufs=2)
            for c in range(CT):
                nc.tensor.matmul(ps[:], w1[c][:, ft * 128 : (ft + 1) * 128],
                                 ln2[c][:], start=(c == 0), stop=(c == CT - 1))
            e = pscr.tile([128, 512], F32, name="e1", tag="er", bufs=4)
            nc.scalar.activation(e[:], ps[:], AF.Exp, bias=vcol(ft, V_B1), scale=1.0)
            r = pscr.tile([128, 512], F32, name="r1", tag="er", bufs=4)
            nc.scalar.activation(r[:], ps[:], AF.Relu, bias=vcol(ft, V_B1), scale=1.0)
            nc.vector.tensor_scalar(out=e[:], in0=e[:], scalar1=-1.0, scalar2=0.0,
                                    op0=ALU.add, op1=ALU.min)
            h1_ft = ph1.tile([128, 512], BF16, name="h1_ft", tag="h1", bufs=8)
            nc.vector.tensor_add(h1_ft[:], r[:], e[:])
            h1.append(h1_ft)

        w2 = []
        for c in range(CT):
            w2_c = pw.tile([128, C], BF16, name="w2_c", tag="wfull", bufs=8)
            nc.sync.dma_start(w2_c[:], w2T[c * 128 : (c + 1) * 128, :])
            w2.append(w2_c)
        for ft in range(CT):
            ps = psA.tile([128, 512], F32, name="psf2", tag="psA", bufs=2)
            for c in range(CT):
                nc.tensor.matmul(ps[:], w2[c][:, ft * 128 : (ft + 1) * 128],
                                 h1[c][:], start=(c == 0), stop=(c == CT - 1))
            e = pscr.tile([128, 512], F32, name="e2", tag="er", bufs=4)
            nc.scalar.activation(e[:], ps[:], AF.Exp, bias=vcol(ft, V_B2), scale=1.0)
            r = pscr.tile([128, 512], F32, name="r2", tag="er", bufs=4)
            nc.scalar.activation(r[:], ps[:], AF.Relu, bias=vcol(ft, V_B2), scale=1.0)
            nc.vector.tensor_scalar(out=e[:], in0=e[:], scalar1=-1.0, scalar2=0.0,
                                    op0=ALU.add, op1=ALU.min)
            nc.vector.tensor_add(r[:], r[:], e[:])  # elu
            nc.vector.tensor_scalar(out=r[:], in0=r[:], scalar1=vcol(ft, V_BNS),
                                    scalar2=vcol(ft, V_BNB), op0=ALU.mult, op1=ALU.add)
            out_ft = pscr.tile([128, 512], F32, name="out_ft", tag="outst", bufs=2)
            nc.vector.tensor_add(out_ft[:], r[:], x2[ft][:])
            nc.sync.dma_start(outT[ft * 128 : (ft + 1) * 128, tsl], out_ft[:])


def build_nc(iters: int = 1):
    nc = bacc.Bacc("TRN2", target_bir_lowering=False, debug=False,
                   num_devices=N_CORES)
    xT = nc.dram_tensor("xT", [C, NT], F32, kind="ExternalInput")
    wqkvT = nc.dram_tensor("wqkvT", [C, 3 * C], BF16, kind="ExternalInput")
    wprojT = nc.dram_tensor("wprojT", [C, C], BF16, kind="ExternalInput")
    w1T = nc.dram_tensor("w1T", [C, C], BF16, kind="ExternalInput")
    w2T = nc.dram_tensor("w2T", [C, C], BF16, kind="ExternalInput")
    vecs = nc.dram_tensor("vecs", [C, 9], F32, kind="ExternalInput")
    ones = nc.dram_tensor("ones", [128, 128], BF16, kind="ExternalInput")
    outT = nc.dram_tensor("outT", [C, NO], F32, kind="ExternalOutput")
    dram = (xT.ap(), wqkvT.ap(), wprojT.ap(), w1T.ap(), w2T.ap(), vecs.ap(),
            ones.ap(), outT.ap())

    from contextlib import ExitStack

    with tile.TileContext(nc) as tc, ExitStack() as ctx:
        pconst = ctx.enter_context(tc.tile_pool(name="pconst", bufs=1))
        pmean = ctx.enter_context(tc.tile_pool(name="pmean", bufs=2))
        pxch = ctx.enter_context(tc.tile_pool(name="pxch", bufs=4))
        pxt = ctx.enter_context(tc.tile_pool(name="pxt", bufs=2))
        phT = ctx.enter_context(tc.tile_pool(name="phT", bufs=8))
        pqkv = ctx.enter_context(tc.tile_pool(name="pqkv", bufs=1))
        pw = ctx.enter_context(tc.tile_pool(name="pw", bufs=1))
        pscr = ctx.enter_context(tc.tile_pool(name="pscr", bufs=1))
        px2 = ctx.enter_context(tc.tile_pool(name="px2", bufs=8))
        ph1 = ctx.enter_context(tc.tile_pool(name="ph1", bufs=8))
        pxo = ctx.enter_context(tc.tile_pool(name="pxo", bufs=2))
        poT = ctx.enter_context(tc.tile_pool(name="poT", bufs=8))
        psA = ctx.enter_context(tc.tile_pool(name="psA", bufs=2, space="PSUM"))
        psS = ctx.enter_context(tc.tile_pool(name="psS", bufs=2, space="PSUM"))
        psR = ctx.enter_context(tc.tile_pool(name="psR", bufs=2, space="PSUM"))
        psO = ctx.enter_context(tc.tile_pool(name="psO", bufs=2, space="PSUM"))
        pools = (pconst, pmean, pxch, pxt, phT, pqkv, pw, pscr, px2, ph1, pxo,
                 poT, psA, psS, psR, psO)
        if iters == 1:
            emit_body(nc, tc, ctx, pools, dram)
        else:
            with tc.For_i(0, iters, 1):
                emit_body(nc, tc, ctx, pools, dram)
    nc.compile()
    return nc


_NC_CACHE = {}


def _get_nc(iters=1):
    if iters not in _NC_CACHE:
        _NC_CACHE[iters] = build_nc(iters)
    return _NC_CACHE[iters]


def make_in_maps(inputs):
    x = np.asarray(inputs["x"], np.float32)
    wqkvT = np.ascontiguousarray(np.asarray(inputs["w_qkv"]).T).astype(ml_dtypes.bfloat16)
    wprojT = np.ascontiguousarray(np.asarray(inputs["w_proj"]).T).astype(ml_dtypes.bfloat16)
    w1T = np.ascontiguousarray(np.asarray(inputs["w1"]).T).astype(ml_dtypes.bfloat16)
    w2T = np.ascontiguousarray(np.asarray(inputs["w2"]).T).astype(ml_dtypes.bfloat16)
    bnscale = (np.asarray(inputs["bn_g"]) /
               np.sqrt(np.asarray(inputs["bn_var"]) + EPS)).astype(np.float32)
    bnbias = (np.asarray(inputs["bn_b"]) -
              np.asarray(inputs["bn_mean"]) * bnscale).astype(np.float32)
    vecs = np.stack([
        np.asarray(inputs["ln1_g"]), np.asarray(inputs["ln1_b"]),
        np.asarray(inputs["ln2_g"]), np.asarray(inputs["ln2_b"]),
        np.asarray(inputs["b_proj"]), np.asarray(inputs["b1"]),
        np.asarray(inputs["b2"]), bnscale, bnbias,
    ], axis=1).astype(np.float32)
    ones = np.ones((128, 128), ml_dtypes.bfloat16)

    in_maps = []
    for core in range(N_CORES):
        b, half = core // 2, core % 2
        xt = x[b].T  # [C, NT]
        if half == 1:
            xt = np.concatenate([xt[:, NO:], xt[:, :NO]], axis=1)
        in_maps.append({
            "xT": np.ascontiguousarray(xt),
            "wqkvT": wqkvT, "wprojT": wprojT, "w1T": w1T, "w2T": w2T,
            "vecs": vecs, "ones": ones,
        })
    return in_maps


def assemble_output(results):
    out = np.empty((B, N, C), np.float32)
    for core in range(N_CORES):
        b, half = core // 2, core % 2
        out[b, half * NO : (half + 1) * NO, :] = results[core]["outT"].T
    return out


def kernel(**inputs):
    nc = _get_nc(1)
    res = run_bass_kernel_spmd(nc, make_in_maps(inputs), list(range(N_CORES)))
    return assemble_output(res.results)
